# revision 15
# baseline (speedup 1.0000x reference)
"""Trainium2 Bass kernel: autoregressive 2-layer GRU decoder (13 steps).

Strategy (per core, batch-sharded 1024/8 = 128):
- Everything lives transposed on-chip: [feature -> partitions, batch -> free].
- The 13 autoregressive decode chains are batched along a diagonal wavefront:
  at wavefront step tau, chain k processes window position j = tau - k.  All
  active chains at a given tau read the SAME window element (x_tau or
  y_{tau-13}); it is replicated (x8-scaled, fp8) across the chain block once
  per tau and its projection rides the gate matmul PSUM accumulation.
- All gate matmuls run fp8e4 DoubleRow (2 K-chunks per instruction, ~1.7x
  bf16 column rate).  Weights and moving operands are pre-scaled by 8, the
  ScalarE activation de-scales with scale=1/64.
- h state is kept in bf16 (accurate update path, 4x DVE rate via STT ops)
  with an x8 fp8 mirror for the matmuls.
- Layer-0 r/z/n input biases ride the input projection via a ones-row in the
  padded second Y-chunk of the replicated window element.
- Fresh chains (j=0, h=0) are handled uniformly: their h slots (bf16 + fp8)
  are zeroed on GpSimd, so recurrent matmuls and updates need no special
  cases.
- n-gate scalar_tensor_tensor ops run on GpSimd (otherwise idle); h updates
  are 3 merged 4x-rate STT ops on DVE across all 4 H-chunks.
"""
import numpy as np

B, T, Y, H = 1024, 13, 188, 512
H3 = 3 * H
NCORE = 8
BS = B // NCORE          # 128 batch rows per core
HC = H // 128            # 4 H chunks
Y1 = Y - 128             # 60 rows in second Y chunk
CPB = 4                  # chains per N-block (4*128 = 512 cols per matmul)
SC = 8.0                 # fp8 operand scale; PSUM carries 64x, ACT undoes

# fp8 weight wall: 84 DoubleRow blocks of [128, 2, 128]
# block index = family offset + ...
BLK_IN0 = 0              # 12 blocks: l0 input proj (m)      kpair=(Ychunk0,1)
BLK_REC0 = 12            # 24 blocks: l0 recurrent (m, kp)
BLK_IN1 = 36             # 24 blocks: l1 input proj (m, kp)
BLK_REC1 = 60            # 24 blocks: l1 recurrent (m, kp)
NBLK = 84

# bias table columns (f32): 0-7 b_rz1 (natural), 8-11 64*b_hn0,
# 12-15 64*b_hn1, 16-19 64*b_in1, 20-21 b_out
NBI = 22

_CACHE = {}
_last_in_maps = None


def _build():
    from contextlib import ExitStack
    import concourse.tile as tile
    from concourse import bacc, mybir

    F32 = mybir.dt.float32
    BF16 = mybir.dt.bfloat16
    FP8 = mybir.dt.float8e4
    AF = mybir.ActivationFunctionType
    OP = mybir.AluOpType
    DR = mybir.MatmulPerfMode.DoubleRow
    IDR = 1.0 / 64.0

    nc = bacc.Bacc("TRN2", target_bir_lowering=False, debug=False)
    xt = nc.declare_dram_parameter("xt", [Y, T, BS], F32, isOutput=False)
    # two complementary fp8 quantizations of the weights, alternated by
    # wavefront-step parity: copy B is the reflected rounding of copy A
    # ((A+B)/2 ~ W), so the per-step systematic quantization error cancels
    # along each chain instead of accumulating linearly.
    wf8 = nc.declare_dram_parameter("wf8", [128, 2, NBLK, 2, 128], FP8,
                                    isOutput=False)
    wbf = nc.declare_dram_parameter("wbf", [128, HC, Y], BF16, isOutput=False)
    bi = nc.declare_dram_parameter("bi", [128, NBI], F32, isOutput=False)
    out = nc.declare_dram_parameter("out", [T, Y, BS], F32, isOutput=True)

    with tile.TileContext(nc) as tc, ExitStack() as ctx:
        wp = ctx.enter_context(tc.tile_pool(name="w", bufs=1))
        hp = ctx.enter_context(tc.tile_pool(name="h", bufs=1))
        ep = ctx.enter_context(tc.tile_pool(name="ep", bufs=4))
        erp = ctx.enter_context(tc.tile_pool(name="erp", bufs=3))
        gp = ctx.enter_context(tc.tile_pool(name="g", bufs=3))
        psP = ctx.enter_context(tc.tile_pool(name="psP", bufs=3, space="PSUM"))
        psO = ctx.enter_context(tc.tile_pool(name="psO", bufs=1, space="PSUM"))

        # ---------------- weights / constants ----------------
        wf8_t = wp.tile([128, 2, NBLK, 2, 128], FP8, tag="wf8", name="wf8")
        nc.sync.dma_start(wf8_t[:, :, :, :, :], wf8[:, :, :, :, :])
        wbf_t = wp.tile([128, HC, Y], BF16, tag="wbf", name="wbf")
        nc.sync.dma_start(wbf_t[:, :, :], wbf[:, :, :])
        bi_t = wp.tile([128, NBI], F32, tag="bi", name="bi")
        nc.sync.dma_start(bi_t[:, :], bi[:, :])

        # ---------------- persistent state ----------------
        hbf = hp.tile([128, 2, HC, T, BS], BF16, tag="hbf", name="hbf")
        hf8 = hp.tile([128, 2, HC, T, BS], FP8, tag="hf8", name="hf8")
        h0bf, h1bf = hbf[:, 0], hbf[:, 1]
        h0f8, h1f8 = hf8[:, 0], hf8[:, 1]

        def make_erep(et):
            """Window element f32 [128,2,BS] -> x8 fp8 replicated x CPB.

            Chunk-1 pad rows are zeroed with a ones bias-row at p64; the
            casts then overwrite the valid rows (0..Y1).  The memsets
            depend only on buffer availability, not on et, so they stay
            off the y -> erep -> matmul critical path.
            """
            er = erp.tile([128, 2, CPB, BS], FP8, tag="er", name="er")
            nc.gpsimd.memset(er[:, 1, :, :], 0.0)
            nc.gpsimd.memset(er[64:65, 1, :, :], 1.0)
            nc.vector.tensor_scalar(
                er[:, 0, :, :],
                et[:, 0:1, :].broadcast_to((128, CPB, BS)), SC, None, OP.mult)
            nc.vector.tensor_scalar(
                er[:Y1, 1, :, :],
                et[:Y1, 1:2, :].broadcast_to((Y1, CPB, BS)), SC, None, OP.mult)
            return er

        def gru_phase(layer, tau, k0, k1, er):
            """One wavefront step of a GRU layer over chains k0..k1."""
            h_bf = h0bf if layer == 0 else h1bf
            h_f8 = h0f8 if layer == 0 else h1f8
            wpar = wf8_t[:, tau % 2]
            A = k1 - k0 + 1
            for b0 in range(0, A, CPB):
                ch = min(CPB, A - b0)
                ks = k0 + b0

                def in_proj(psj, m, stop):
                    if layer == 0:
                        nc.tensor.matmul(
                            psj, wpar[:, BLK_IN0 + m],
                            er[:, :, :ch, :],
                            start=True, stop=stop, perf_mode=DR)
                    else:
                        for kp in range(2):
                            nc.tensor.matmul(
                                psj, wpar[:, BLK_IN1 + 2 * m + kp],
                                h0f8[:, 2 * kp:2 * kp + 2, ks:ks + ch, :],
                                start=(kp == 0), stop=(stop and kp == 1),
                                perf_mode=DR)

                def rec_proj(psj, m, start):
                    blk = BLK_REC0 if layer == 0 else BLK_REC1
                    for kp in range(2):
                        nc.tensor.matmul(
                            psj, wpar[:, blk + 2 * m + kp],
                            h_f8[:, 2 * kp:2 * kp + 2, ks:ks + ch, :],
                            start=(start and kp == 0), stop=(kp == 1),
                            perf_mode=DR)

                rt = gp.tile([128, 4, CPB, BS], BF16, tag="rt", name="rt")
                zt = gp.tile([128, 4, CPB, BS], BF16, tag="zt", name="zt")
                for pi in range(4):  # r (pi 0,1) and z (pi 2,3) pairs
                    ps = psP.tile([128, 2, CPB, BS], F32, tag="ps", name="ps")
                    for j in range(2):
                        m = 2 * pi + j
                        in_proj(ps[:, j, :ch, :], m, False)
                        rec_proj(ps[:, j, :ch, :], m, False)
                    dst = rt if pi < 2 else zt
                    di = (2 * pi) % 4
                    if layer == 0:
                        nc.scalar.activation(dst[:, di:di + 2, :ch, :],
                                             ps[:, :, :ch, :], AF.Sigmoid,
                                             scale=IDR)
                    else:
                        for j in range(2):
                            m = 2 * pi + j
                            nc.scalar.activation(
                                dst[:, di + j, :ch, :], ps[:, j, :ch, :],
                                AF.Sigmoid, bias=bi_t[:, m:m + 1], scale=IDR)
                # n gates
                tmp = gp.tile([128, 4, CPB, BS], F32, tag="tmp", name="tmp",
                              bufs=2)
                npre = gp.tile([128, 4, CPB, BS], F32, tag="npre", name="npre",
                               bufs=2)
                bhn_c = 8 if layer == 0 else 12
                for pi in range(2):
                    gps = psP.tile([128, 2, CPB, BS], F32, tag="ps", name="ps")
                    for j in range(2):
                        i = 2 * pi + j
                        rec_proj(gps[:, j, :ch, :], 8 + i, True)
                        nc.vector.scalar_tensor_tensor(
                            tmp[:, i, :ch, :], gps[:, j, :ch, :],
                            bi_t[:, bhn_c + i:bhn_c + i + 1],
                            rt[:, i, :ch, :], OP.add, OP.mult)
                    xps = psP.tile([128, 2, CPB, BS], F32, tag="ps", name="ps")
                    for j in range(2):
                        i = 2 * pi + j
                        in_proj(xps[:, j, :ch, :], 8 + i, True)
                    if layer == 0:
                        nc.vector.tensor_add(
                            npre[:, 2 * pi:2 * pi + 2, :ch, :],
                            xps[:, :, :ch, :], tmp[:, 2 * pi:2 * pi + 2, :ch, :])
                    else:
                        for j in range(2):
                            i = 2 * pi + j
                            nc.vector.scalar_tensor_tensor(
                                npre[:, i, :ch, :], xps[:, j, :ch, :],
                                bi_t[:, 16 + i:16 + i + 1],
                                tmp[:, i, :ch, :], OP.add, OP.add)
                nt = gp.tile([128, 4, CPB, BS], BF16, tag="nt", name="nt")
                nc.scalar.activation(nt[:, :, :ch, :], npre[:, :, :ch, :],
                                     AF.Tanh, scale=IDR)
                # h <- n + z*(h - n), merged over all 4 H chunks (4x STT)
                hs = h_bf[:, :, ks:ks + ch, :]
                d = gp.tile([128, 4, CPB, BS], BF16, tag="d", name="d",
                            bufs=2)
                nc.vector.scalar_tensor_tensor(
                    d[:, :, :ch, :], hs, 1.0, nt[:, :, :ch, :],
                    OP.mult, OP.subtract)
                nc.vector.scalar_tensor_tensor(
                    d[:, :, :ch, :], d[:, :, :ch, :], 1.0, zt[:, :, :ch, :],
                    OP.mult, OP.mult)
                nc.vector.scalar_tensor_tensor(
                    hs, d[:, :, :ch, :], 1.0, nt[:, :, :ch, :],
                    OP.mult, OP.add)
                # fp8 mirror (x8) for next-step matmuls (GpSimd: SBUF-only,
                # keeps DVE free for the PSUM-reading gate ops)
                nc.gpsimd.tensor_scalar(
                    h_f8[:, :, ks:ks + ch, :], hs, SC, None, OP.mult)

        def out_phase(tau, elems):
            """Emit y_{tau-13} = W_out @ relu(h1) + b_out + residual."""
            c = tau - 13
            rl = gp.tile([128, HC, BS], BF16, tag="rl", name="rl", bufs=2)
            nc.scalar.activation(rl[:, :, :], h1bf[:, :, c, :], AF.Relu)
            po = psO.tile([128, 2, BS], F32, tag="po", name="po")
            for kc in range(HC):
                nc.tensor.matmul(po[:, 0, :], wbf_t[:, kc, 0:128],
                                 rl[:, kc, :],
                                 start=(kc == 0), stop=(kc == HC - 1))
            for kc in range(HC):
                nc.tensor.matmul(po[:Y1, 1, :], wbf_t[:, kc, 128:Y],
                                 rl[:, kc, :],
                                 start=(kc == 0), stop=(kc == HC - 1))
            eres = elems[c + 12]
            y = ep.tile([128, 2, BS], F32, tag="e", name="y")
            nc.vector.scalar_tensor_tensor(
                y[:, 0, :], po[:, 0, :], bi_t[:, 20:21], eres[:, 0, :],
                OP.add, OP.add)
            nc.vector.scalar_tensor_tensor(
                y[:Y1, 1, :], po[:Y1, 1, :], bi_t[:Y1, 21:22],
                eres[:Y1, 1, :], OP.add, OP.add)
            nc.sync.dma_start(out[c, 0:128, :], y[:, 0, :])
            nc.sync.dma_start(out[c, 128:Y, :], y[:Y1, 1, :])
            return y

        # ---------------- wavefront ----------------
        elems = {}
        for tau in range(26):
            if tau <= 12:
                et = ep.tile([128, 2, BS], F32, tag="e", name="e")
                nc.sync.dma_start(et[:, 0, :], xt[0:128, tau, :])
                nc.sync.dma_start(et[:Y1, 1, :], xt[128:Y, tau, :])
                elems[tau] = et
            # zero fresh-chain h slots (uniform blocks need h=0 readable)
            if tau <= 12:  # l0 fresh chain = tau
                nc.gpsimd.memset(h0bf[:, :, tau, :], 0.0)
                nc.gpsimd.memset(h0f8[:, :, tau, :], 0.0)
            if 1 <= tau <= 13:  # l1 fresh chain = tau-1
                nc.gpsimd.memset(h1bf[:, :, tau - 1, :], 0.0)
                nc.gpsimd.memset(h1f8[:, :, tau - 1, :], 0.0)
            k0, k1 = max(0, tau - 13), min(T - 1, tau - 1)
            if k0 <= k1:
                gru_phase(1, tau, k0, k1, None)
            if tau >= 13:
                elems[tau] = out_phase(tau, elems)
            if tau <= 24:
                k0, k1 = max(0, tau - 12), min(T - 1, tau)
                er = make_erep(elems[tau])
                gru_phase(0, tau, k0, k1, er)

    nc.finalize()
    return nc


def _prep_in_maps(inputs):
    import ml_dtypes
    x = np.asarray(inputs["x"], np.float32)
    f = lambda k: np.asarray(inputs[k], np.float32)
    b_ih0, b_hh0 = f("b_ih0"), f("b_hh0")
    b_ih1, b_hh1 = f("b_ih1"), f("b_hh1")
    b_out = f("b_out")

    bias_arr = np.zeros((128, NBI), np.float32)
    brz1 = (b_ih1 + b_hh1)[:2 * H]
    for m in range(8):
        bias_arr[:, m] = brz1[m * 128:(m + 1) * 128]
    for i in range(4):
        bias_arr[:, 8 + i] = 64.0 * b_hh0[2 * H + i * 128:2 * H + (i + 1) * 128]
        bias_arr[:, 12 + i] = 64.0 * b_hh1[2 * H + i * 128:2 * H + (i + 1) * 128]
        bias_arr[:, 16 + i] = 64.0 * b_ih1[2 * H + i * 128:2 * H + (i + 1) * 128]
    bias_arr[:, 20] = b_out[:128]
    bias_arr[:Y1, 21] = b_out[128:Y]

    # fp8 DoubleRow weight wall, x8 scale
    wall = np.zeros((128, NBLK, 2, 128), np.float32)
    wih0T = f("W_ih0").T          # [Y, 3H]
    b_rzn0 = np.concatenate([(b_ih0 + b_hh0)[:2 * H], b_ih0[2 * H:]])  # [3H]
    for m in range(12):
        cols = slice(m * 128, (m + 1) * 128)
        blkA = np.zeros((128, 128), np.float32)
        blkB = np.zeros((128, 128), np.float32)
        blkA[:, :] = 8.0 * wih0T[0:128, cols]
        blkB[:Y1, :] = 8.0 * wih0T[128:Y, cols]
        blkB[64, :] = 64.0 * b_rzn0[cols]   # ones-row bias fold (p64)
        wall[:, BLK_IN0 + m, 0] = blkA
        wall[:, BLK_IN0 + m, 1] = blkB
    for nm, base in (("W_hh0", BLK_REC0), ("W_ih1", BLK_IN1),
                     ("W_hh1", BLK_REC1)):
        wT = f(nm).T              # [H, 3H]
        for m in range(12):
            cols = slice(m * 128, (m + 1) * 128)
            for kp in range(2):
                wall[:, base + 2 * m + kp, 0] = \
                    8.0 * wT[kp * 256:kp * 256 + 128, cols]
                wall[:, base + 2 * m + kp, 1] = \
                    8.0 * wT[kp * 256 + 128:kp * 256 + 256, cols]
    wall = np.clip(wall, -240.0, 240.0)
    wa = wall.astype(ml_dtypes.float8_e4m3)
    wb = np.clip(2.0 * wall - wa.astype(np.float32), -240.0, 240.0) \
        .astype(ml_dtypes.float8_e4m3)
    wall2 = np.stack([wa, wb], axis=1)          # [128, 2, NBLK, 2, 128]

    wbf = np.zeros((128, HC, Y), np.float32)
    woutT = f("W_out").T          # [H, Y]
    for kc in range(HC):
        wbf[:, kc, :] = woutT[kc * 128:(kc + 1) * 128]

    base = {
        "wf8": wall2,
        "wbf": wbf.astype(ml_dtypes.bfloat16),
        "bi": bias_arr,
    }
    in_maps = []
    for c in range(NCORE):
        m = dict(base)
        m["xt"] = np.ascontiguousarray(
            x[c * BS:(c + 1) * BS].transpose(2, 1, 0))
        in_maps.append(m)
    return in_maps


def kernel(**inputs):
    global _last_in_maps
    from concourse.bass_utils import run_bass_kernel_spmd
    if "nc" not in _CACHE:
        _CACHE["nc"] = _build()
    in_maps = _prep_in_maps(inputs)
    _last_in_maps = in_maps
    res = run_bass_kernel_spmd(_CACHE["nc"], in_maps, list(range(NCORE)))
    outs = [np.asarray(res.results[i]["out"]).transpose(2, 0, 1)
            for i in range(NCORE)]
    return np.concatenate(outs, axis=0).astype(np.float32)


# revision 22
# speedup vs baseline: 2.1829x; 2.1829x over previous
"""Trainium2 Bass kernel: autoregressive 2-layer GRU decoder (13 steps).

Strategy (per core, batch-sharded 1024/8 = 128):
- Everything lives transposed on-chip: [feature -> partitions, batch -> free].
- The 13 autoregressive decode chains are batched along a diagonal wavefront:
  at wavefront step tau, chain k processes window position j = tau - k.  All
  active chains at a given tau read the SAME window element (x_tau or
  y_{tau-13}); a x8-scaled fp8 replica tile feeds the input projection, which
  rides the gate matmul PSUM accumulation.  For x-elements the replicas are
  precast on the host and DMAed; for y-elements ScalarE casts them.
- All gate matmuls run fp8e4 DoubleRow (2 K-chunks per instruction).  Two
  complementary weight quantizations (B = reflected rounding of A) alternate
  by tau parity so per-step systematic quantization error cancels along each
  chain instead of accumulating linearly.
- h state is bf16 chain-major [128, T, HC, BS] so elementwise slices are
  fully dense (DVE 2x tensor_tensor); an x8 fp8 mirror for the matmuls is
  produced by ScalarE ACT-copy (ScalarE converts fp8 at full rate; DVE and
  GpSimd do not).
- Layer-0 r/z/n input biases ride the input projection via a ones-row in the
  padded second Y-chunk of the window-element replica tile.
- Fresh chains (j=0, h=0) are handled uniformly: their h slots (bf16 + fp8)
  are zeroed on GpSimd, so recurrent matmuls and updates need no special
  cases.
- h updates (3 dense bf16 TTs) alternate per block between DVE and GpSimd to
  balance load; the PSUM-reading n-gate ops stay on DVE.
"""
import numpy as np

B, T, Y, H = 1024, 13, 188, 512
H3 = 3 * H
NCORE = 8
BS = B // NCORE          # 128 batch rows per core
HC = H // 128            # 4 H chunks
Y1 = Y - 128             # 60 rows in second Y chunk
CPB = 4                  # chains per N-block (4*128 = 512 cols per matmul)
SC = 8.0                 # fp8 operand scale; PSUM carries 64x, ACT undoes

# fp8 weight wall: per parity, 84 DoubleRow blocks of [128, 2, 128]
BLK_IN0 = 0              # 12 blocks: l0 input proj (m)      kpair=(Ychunk0,1)
BLK_REC0 = 12            # 24 blocks: l0 recurrent (m, kp)
BLK_IN1 = 36             # 24 blocks: l1 input proj (m, kp)
BLK_REC1 = 60            # 24 blocks: l1 recurrent (m, kp)
NBLK = 84

# bias table columns (f32): 0-7 b_rz1 (natural), 8-11 64*b_hn0,
# 12-15 64*b_hn1, 16-19 64*b_in1, 20-21 b_out
NBI = 22

_CACHE = {}
_last_in_maps = None


def _build():
    from contextlib import ExitStack
    import concourse.tile as tile
    from concourse import bacc, mybir

    F32 = mybir.dt.float32
    BF16 = mybir.dt.bfloat16
    FP8 = mybir.dt.float8e4
    AF = mybir.ActivationFunctionType
    OP = mybir.AluOpType
    DR = mybir.MatmulPerfMode.DoubleRow
    IDR = 1.0 / 64.0

    nc = bacc.Bacc("TRN2", target_bir_lowering=False, debug=False)
    x12 = nc.declare_dram_parameter("x12", [Y, BS], F32, isOutput=False)
    xf8 = nc.declare_dram_parameter("xf8", [T, 128, 2, CPB, BS], FP8,
                                    isOutput=False)
    wf8 = nc.declare_dram_parameter("wf8", [128, 2, NBLK, 2, 128], FP8,
                                    isOutput=False)
    wbf = nc.declare_dram_parameter("wbf", [128, HC, Y], BF16, isOutput=False)
    bi = nc.declare_dram_parameter("bi", [128, NBI], F32, isOutput=False)
    out = nc.declare_dram_parameter("out", [T, Y, BS], F32, isOutput=True)

    with tile.TileContext(nc) as tc, ExitStack() as ctx:
        wp = ctx.enter_context(tc.tile_pool(name="w", bufs=1))
        hp = ctx.enter_context(tc.tile_pool(name="h", bufs=1))
        ep = ctx.enter_context(tc.tile_pool(name="ep", bufs=4))
        erp = ctx.enter_context(tc.tile_pool(name="erp", bufs=3))
        gp = ctx.enter_context(tc.tile_pool(name="g", bufs=3))
        psP = ctx.enter_context(tc.tile_pool(name="psP", bufs=3, space="PSUM"))
        psO = ctx.enter_context(tc.tile_pool(name="psO", bufs=1, space="PSUM"))

        # ---------------- weights / constants ----------------
        wf8_t = wp.tile([128, 2, NBLK, 2, 128], FP8, tag="wf8", name="wf8")
        nc.sync.dma_start(wf8_t[:, :, :, :, :], wf8[:, :, :, :, :])
        wbf_t = wp.tile([128, HC, Y], BF16, tag="wbf", name="wbf")
        nc.sync.dma_start(wbf_t[:, :, :], wbf[:, :, :])
        bi_t = wp.tile([128, NBI], F32, tag="bi", name="bi")
        nc.sync.dma_start(bi_t[:, :], bi[:, :])

        # ---------------- persistent state (chain-major) ----------------
        # bf16 state chain-major (dense DVE slices); fp8 mirror chunk-major
        # (DoubleRow-matmul-friendly [2-chunk, chain, batch] slices)
        hbf = hp.tile([128, 2, T, HC, BS], BF16, tag="hbf", name="hbf")
        hf8 = hp.tile([128, 2, HC, T, BS], FP8, tag="hf8", name="hf8")
        h0bf, h1bf = hbf[:, 0], hbf[:, 1]
        h0f8, h1f8 = hf8[:, 0], hf8[:, 1]
        upd_flip = [0]

        def gru_phase(layer, tau, k0, k1, er):
            """One wavefront step of a GRU layer over chains k0..k1."""
            h_bf = h0bf if layer == 0 else h1bf
            h_f8 = h0f8 if layer == 0 else h1f8
            wpar = wf8_t[:, tau % 2]
            A = k1 - k0 + 1
            for b0 in range(0, A, CPB):
                ch = min(CPB, A - b0)
                ks = k0 + b0

                def in_proj(psj, m, stop):
                    if layer == 0:
                        nc.tensor.matmul(
                            psj, wpar[:, BLK_IN0 + m],
                            er[:, :, :ch, :],
                            start=True, stop=stop, perf_mode=DR)
                    else:
                        for kp in range(2):
                            nc.tensor.matmul(
                                psj, wpar[:, BLK_IN1 + 2 * m + kp],
                                h0f8[:, 2 * kp:2 * kp + 2, ks:ks + ch, :],
                                start=(kp == 0), stop=(stop and kp == 1),
                                perf_mode=DR)

                def rec_proj(psj, m, start):
                    blk = BLK_REC0 if layer == 0 else BLK_REC1
                    for kp in range(2):
                        nc.tensor.matmul(
                            psj, wpar[:, blk + 2 * m + kp],
                            h_f8[:, 2 * kp:2 * kp + 2, ks:ks + ch, :],
                            start=(start and kp == 0), stop=(kp == 1),
                            perf_mode=DR)

                rt = gp.tile([128, CPB, 4, BS], BF16, tag="rt", name="rt")
                zt = gp.tile([128, CPB, 4, BS], BF16, tag="zt", name="zt")
                for pi in range(4):  # r (pi 0,1) and z (pi 2,3) pairs
                    ps = psP.tile([128, 2, CPB, BS], F32, tag="ps", name="ps")
                    for j in range(2):
                        m = 2 * pi + j
                        in_proj(ps[:, j, :ch, :], m, False)
                        rec_proj(ps[:, j, :ch, :], m, False)
                    dst = rt if pi < 2 else zt
                    di = (2 * pi) % 4
                    if layer == 0:
                        nc.scalar.activation(
                            dst[:, :ch, di:di + 2, :].transpose([0, 2, 1, 3]),
                            ps[:, :, :ch, :], AF.Sigmoid, scale=IDR)
                    else:
                        for j in range(2):
                            m = 2 * pi + j
                            nc.scalar.activation(
                                dst[:, :ch, di + j, :], ps[:, j, :ch, :],
                                AF.Sigmoid, bias=bi_t[:, m:m + 1], scale=IDR)
                # n gates (64-scaled f32 until the merged tanh)
                tmp = gp.tile([128, CPB, 4, BS], F32, tag="tmp", name="tmp",
                              bufs=2)
                npre = gp.tile([128, CPB, 4, BS], F32, tag="npre", name="npre",
                               bufs=2)
                bhn_c = 8 if layer == 0 else 12
                for pi in range(2):
                    gps = psP.tile([128, 2, CPB, BS], F32, tag="ps", name="ps")
                    for j in range(2):
                        i = 2 * pi + j
                        rec_proj(gps[:, j, :ch, :], 8 + i, True)
                        nc.vector.scalar_tensor_tensor(
                            tmp[:, :ch, i, :], gps[:, j, :ch, :],
                            bi_t[:, bhn_c + i:bhn_c + i + 1],
                            rt[:, :ch, i, :], OP.add, OP.mult)
                    xps = psP.tile([128, 2, CPB, BS], F32, tag="ps", name="ps")
                    for j in range(2):
                        i = 2 * pi + j
                        in_proj(xps[:, j, :ch, :], 8 + i, True)
                    if layer == 0:
                        nc.vector.tensor_add(
                            npre[:, :ch, 2 * pi:2 * pi + 2, :]
                            .transpose([0, 2, 1, 3]),
                            xps[:, :, :ch, :],
                            tmp[:, :ch, 2 * pi:2 * pi + 2, :]
                            .transpose([0, 2, 1, 3]))
                    else:
                        for j in range(2):
                            i = 2 * pi + j
                            nc.vector.scalar_tensor_tensor(
                                npre[:, :ch, i, :], xps[:, j, :ch, :],
                                bi_t[:, 16 + i:16 + i + 1],
                                tmp[:, :ch, i, :], OP.add, OP.add)
                nt = gp.tile([128, CPB, 4, BS], BF16, tag="nt", name="nt")
                nc.scalar.activation(nt[:, :ch, :, :], npre[:, :ch, :, :],
                                     AF.Tanh, scale=IDR)
                # h <- n + z*(h - n): 3 dense bf16 TTs, alternate DVE/GpSimd
                hs = h_bf[:, ks:ks + ch, :, :]
                d = gp.tile([128, CPB, 4, BS], BF16, tag="d", name="d",
                            bufs=2)
                eng = nc.vector if upd_flip[0] % 2 == 0 else nc.gpsimd
                upd_flip[0] += 1
                eng.tensor_sub(d[:, :ch, :, :], hs, nt[:, :ch, :, :])
                eng.tensor_mul(d[:, :ch, :, :], d[:, :ch, :, :],
                               zt[:, :ch, :, :])
                eng.tensor_add(hs, d[:, :ch, :, :], nt[:, :ch, :, :])
                # x8 fp8 mirror on ScalarE (full-rate fp8 conversion);
                # output AP transposes chain-major -> chunk-major
                nc.scalar.activation(
                    h_f8[:, :, ks:ks + ch, :].transpose([0, 2, 1, 3]),
                    hs, AF.Copy, scale=SC)

        def out_phase(tau, elems):
            """Emit y_{tau-13} = W_out @ relu(h1) + b_out + residual."""
            c = tau - 13
            rl = gp.tile([128, HC, BS], BF16, tag="rl", name="rl", bufs=2)
            nc.scalar.activation(rl[:, :, :], h1bf[:, c, :, :], AF.Relu)
            po = psO.tile([128, 2, BS], F32, tag="po", name="po")
            for kc in range(HC):
                nc.tensor.matmul(po[:, 0, :], wbf_t[:, kc, 0:128],
                                 rl[:, kc, :],
                                 start=(kc == 0), stop=(kc == HC - 1))
            for kc in range(HC):
                nc.tensor.matmul(po[:Y1, 1, :], wbf_t[:, kc, 128:Y],
                                 rl[:, kc, :],
                                 start=(kc == 0), stop=(kc == HC - 1))
            eres = elems[c + 12]
            y = ep.tile([128, 2, BS], F32, tag="e", name="y")
            nc.vector.scalar_tensor_tensor(
                y[:, 0, :], po[:, 0, :], bi_t[:, 20:21], eres[:, 0, :],
                OP.add, OP.add)
            nc.vector.scalar_tensor_tensor(
                y[:Y1, 1, :], po[:Y1, 1, :], bi_t[:Y1, 21:22],
                eres[:Y1, 1, :], OP.add, OP.add)
            nc.sync.dma_start(out[c, 0:128, :], y[:, 0, :])
            nc.sync.dma_start(out[c, 128:Y, :], y[:Y1, 1, :])
            return y

        # ---------------- wavefront ----------------
        elems = {}
        for tau in range(26):
            if tau == 12:
                et = ep.tile([128, 2, BS], F32, tag="e", name="e")
                nc.sync.dma_start(et[:, 0, :], x12[0:128, :])
                nc.sync.dma_start(et[:Y1, 1, :], x12[128:Y, :])
                elems[tau] = et
            # zero fresh-chain h slots (uniform blocks need h=0 readable)
            if tau <= 12:  # l0 fresh chain = tau
                nc.gpsimd.memset(h0bf[:, tau, :, :], 0.0)
                nc.gpsimd.memset(h0f8[:, :, tau, :], 0.0)
            if 1 <= tau <= 13:  # l1 fresh chain = tau-1
                nc.gpsimd.memset(h1bf[:, tau - 1, :, :], 0.0)
                nc.gpsimd.memset(h1f8[:, :, tau - 1, :], 0.0)
            k0, k1 = max(0, tau - 13), min(T - 1, tau - 1)
            if k0 <= k1:
                gru_phase(1, tau, k0, k1, None)
            if tau >= 13:
                elems[tau] = out_phase(tau, elems)
            if tau <= 24:
                er = erp.tile([128, 2, CPB, BS], FP8, tag="er", name="er")
                if tau <= 12:
                    # host-precast x8 fp8 replicas
                    nc.sync.dma_start(er[:, :, :, :], xf8[tau])
                else:
                    # y element: 4 per-replica ScalarE casts (x8 -> fp8);
                    # pad rows of chunk 1 carry zeros + the ones bias row
                    # pad rows [Y1:128) of chunk 1: zeros + ones row at p64
                    # (base partition must be in {0,32,64,96}, <=32 from 32)
                    nc.gpsimd.memset(er[32:64, 1, :, :], 0.0)
                    nc.gpsimd.memset(er[64:, 1, :, :], 0.0)
                    nc.gpsimd.memset(er[64:65, 1, :, :], 1.0)
                    y = elems[tau]
                    for c in range(CPB):
                        nc.scalar.activation(er[:, 0, c, :], y[:, 0, :],
                                             AF.Copy, scale=SC)
                        nc.scalar.activation(er[:Y1, 1, c, :], y[:Y1, 1, :],
                                             AF.Copy, scale=SC)
                k0, k1 = max(0, tau - 12), min(T - 1, tau)
                gru_phase(0, tau, k0, k1, er)

    nc.finalize()
    return nc


def _prep_in_maps(inputs):
    import ml_dtypes
    x = np.asarray(inputs["x"], np.float32)
    f = lambda k: np.asarray(inputs[k], np.float32)
    b_ih0, b_hh0 = f("b_ih0"), f("b_hh0")
    b_ih1, b_hh1 = f("b_ih1"), f("b_hh1")
    b_out = f("b_out")

    bias_arr = np.zeros((128, NBI), np.float32)
    brz1 = (b_ih1 + b_hh1)[:2 * H]
    for m in range(8):
        bias_arr[:, m] = brz1[m * 128:(m + 1) * 128]
    for i in range(4):
        bias_arr[:, 8 + i] = 64.0 * b_hh0[2 * H + i * 128:2 * H + (i + 1) * 128]
        bias_arr[:, 12 + i] = 64.0 * b_hh1[2 * H + i * 128:2 * H + (i + 1) * 128]
        bias_arr[:, 16 + i] = 64.0 * b_ih1[2 * H + i * 128:2 * H + (i + 1) * 128]
    bias_arr[:, 20] = b_out[:128]
    bias_arr[:Y1, 21] = b_out[128:Y]

    # fp8 DoubleRow weight wall, x8 scale
    wall = np.zeros((128, NBLK, 2, 128), np.float32)
    wih0T = f("W_ih0").T          # [Y, 3H]
    b_rzn0 = np.concatenate([(b_ih0 + b_hh0)[:2 * H], b_ih0[2 * H:]])  # [3H]
    for m in range(12):
        cols = slice(m * 128, (m + 1) * 128)
        blkA = np.zeros((128, 128), np.float32)
        blkB = np.zeros((128, 128), np.float32)
        blkA[:, :] = 8.0 * wih0T[0:128, cols]
        blkB[:Y1, :] = 8.0 * wih0T[128:Y, cols]
        blkB[64, :] = 64.0 * b_rzn0[cols]   # ones-row bias fold (p64)
        wall[:, BLK_IN0 + m, 0] = blkA
        wall[:, BLK_IN0 + m, 1] = blkB
    for nm, base in (("W_hh0", BLK_REC0), ("W_ih1", BLK_IN1),
                     ("W_hh1", BLK_REC1)):
        wT = f(nm).T              # [H, 3H]
        for m in range(12):
            cols = slice(m * 128, (m + 1) * 128)
            for kp in range(2):
                wall[:, base + 2 * m + kp, 0] = \
                    8.0 * wT[kp * 256:kp * 256 + 128, cols]
                wall[:, base + 2 * m + kp, 1] = \
                    8.0 * wT[kp * 256 + 128:kp * 256 + 256, cols]
    wall = np.clip(wall, -240.0, 240.0)
    wa = wall.astype(ml_dtypes.float8_e4m3)
    wb = np.clip(2.0 * wall - wa.astype(np.float32), -240.0, 240.0) \
        .astype(ml_dtypes.float8_e4m3)
    wall2 = np.stack([wa, wb], axis=1)          # [128, 2, NBLK, 2, 128]

    wbf = np.zeros((128, HC, Y), np.float32)
    woutT = f("W_out").T          # [H, Y]
    for kc in range(HC):
        wbf[:, kc, :] = woutT[kc * 128:(kc + 1) * 128]

    base = {
        "wf8": wall2,
        "wbf": wbf.astype(ml_dtypes.bfloat16),
        "bi": bias_arr,
    }
    in_maps = []
    for c in range(NCORE):
        m = dict(base)
        xc = np.ascontiguousarray(
            x[c * BS:(c + 1) * BS].transpose(2, 1, 0))      # [Y, T, BS]
        m["x12"] = np.ascontiguousarray(xc[:, 12, :])
        # host-precast x8 fp8 window replicas [T, 128, 2, CPB, BS]
        xr = np.zeros((T, 128, 2, CPB, BS), np.float32)
        for t in range(T):
            xr[t, :, 0, :, :] = 8.0 * xc[0:128, t, :][:, None, :]
            xr[t, :Y1, 1, :, :] = 8.0 * xc[128:Y, t, :][:, None, :]
            xr[t, 64, 1, :, :] = 1.0    # ones bias row
        m["xf8"] = xr.astype(ml_dtypes.float8_e4m3)
        in_maps.append(m)
    return in_maps


def kernel(**inputs):
    global _last_in_maps
    from concourse.bass_utils import run_bass_kernel_spmd
    if "nc" not in _CACHE:
        _CACHE["nc"] = _build()
    in_maps = _prep_in_maps(inputs)
    _last_in_maps = in_maps
    res = run_bass_kernel_spmd(_CACHE["nc"], in_maps, list(range(NCORE)))
    outs = [np.asarray(res.results[i]["out"]).transpose(2, 0, 1)
            for i in range(NCORE)]
    return np.concatenate(outs, axis=0).astype(np.float32)
